# revision 17
# baseline (speedup 1.0000x reference)
"""Causal self-attention (B=8, T=1024, C=768, H=12, Dh=64) on 8 TRN2 NeuronCores.

Sharding: batch data-parallel. Core b computes the full attention block for
batch element b (weights replicated). No collectives.

Per-core dataflow (all matmuls bf16; fp32 weights cast on idle engines so
every stationary operand gets fast bf16 LDWEIGHTS/FWL):
  1. x [T,C] -> bf16 (DVE cast) -> xT [C,T] via PE transposes, ACT evac.
  2. Q^T,K^T [C,T] = W^T @ xT (evac casts to bf16 qkt); V [t, c] = x @ W_v
     (bias-add evac to bf16), stored per head with an all-ones column
     (V_aug [k, 65]) so the P@V matmul also accumulates softmax denominators.
     W_out DMA+cast deferred to mid-attention (preamble is HBM-bound).
  3. Per head h, software-pipelined over k-blocks: S^T(kb) [k=128, q] emitted
     ahead; P^T = exp(S^T/8) (ACT, bf16 out, exact causal spans) one block
     behind; sub-diagonal zeroing via in-place gpsimd affine_select; PV
     (O'^T [65, q] += V_aug^T @ P^T) lags TWO blocks so its mask is always
     ready when the PE reaches it. Row 64 of O' = softmax denominator.
  4. Normalize per 512-wide q-chunk: qc0 on ACT (1/s = exp(-ln s), same pinned
     table), qc1 on DVE (reciprocal_approx_fast); gpsimd partition_broadcast;
     DVE multiply -> OT (bf16). The last two PVs + qc1 normalize of each head
     are deferred into the next head's pipeline.
  5. y [T,C] = OT-as-lhsT @ W_out + b_out, DMA to DRAM.
"""

import numpy as np

B, T, C = 8, 1024, 768
H, D = 12, 64
TB = T // 128  # 8 t/k blocks
CB = C // 128  # 6 channel blocks
J = H // 2  # 6 head pairs
NCORES = 8

_CACHE = {}


def _ensure_path():
    import sys

    for p in ("/opt/trn_rl_repo",):
        if p not in sys.path:
            sys.path.insert(0, p)


def _emit(nc, tc, tile, mybir, make_identity):
    f32 = mybir.dt.float32
    f32r = mybir.dt.float32r
    bf16 = mybir.dt.bfloat16
    Exp = mybir.ActivationFunctionType.Exp
    Ln = mybir.ActivationFunctionType.Ln
    isge = mybir.AluOpType.is_ge

    x_d = nc.dram_tensor("x", [T, C], f32, kind="ExternalInput")
    wqkv_d = nc.dram_tensor("W_qkv", [C, 3 * C], f32r, kind="ExternalInput")
    bqkv_d = nc.dram_tensor("b_qkv", [3 * C], f32, kind="ExternalInput")
    wout_d = nc.dram_tensor("W_out", [C, C], f32r, kind="ExternalInput")
    bout_d = nc.dram_tensor("b_out", [C], f32, kind="ExternalInput")
    y_d = nc.dram_tensor("y_out", [T, C], f32, kind="ExternalOutput")

    with (
        tc.tile_pool(name="const", bufs=1) as const_pool,
        tc.tile_pool(name="wres", bufs=1) as wres,
        tc.tile_pool(name="wqkp", bufs=2) as wqk_pool,
        tc.tile_pool(name="xin", bufs=4) as xin_pool,
        tc.tile_pool(name="big", bufs=1) as big,
        tc.tile_pool(name="qktp", bufs=2) as qkt_pool,
        tc.tile_pool(name="ptp", bufs=6) as pt_pool,
        tc.tile_pool(name="yp", bufs=2) as y_pool,
        tc.tile_pool(name="smallp", bufs=2) as small_pool,
        # PSUM: st 4x[128,512]f32 = 4 banks (S chunks + proj chains +
        # transposes share the rotation); ot 4x[*,512]f32 = 4 banks.
        tc.tile_pool(name="stp", bufs=4, space="PSUM") as st_psum,
        tc.tile_pool(name="op", bufs=4, space="PSUM") as o_psum,
    ):
        ident = const_pool.tile([128, 128], f32, name="ident")
        make_identity(nc, ident[:])

        # x and W_v load at full HWDGE rate as f32 and are cast to bf16 on
        # the (preamble-idle) DVE / ACT engines; wqk and wout use gpsimd
        # SWDGE casting DMAs (lower DMA rate, but off the critical path).
        wv_raw = wres.tile([128, CB, C], f32r, name="wv_raw")
        wv = wres.tile([128, CB, C], bf16, name="wv")

        def wv_dma(half):
            for cb in range(CB):
                nc.scalar.dma_start(
                    wv_raw[:, cb, half * 384 : (half + 1) * 384],
                    wqkv_d[
                        cb * 128 : (cb + 1) * 128,
                        2 * C + half * 384 : 2 * C + (half + 1) * 384,
                    ],
                )

        def wv_cast(half):
            for cb in range(CB):
                nc.scalar.copy(
                    wv[:, cb, half * 384 : (half + 1) * 384],
                    wv_raw[:, cb, half * 384 : (half + 1) * 384].bitcast(f32),
                )

        xT = big.tile([128, CB, T], bf16, name="xT")
        V = big.tile([128, TB, H, D + 1], bf16, name="V")
        OT = [big.tile([128, T], bf16, name=f"OT{j}", tag=f"OT{j}") for j in range(J)]

        # bias DMAs early (HWDGE, cheap); their gpsimd broadcasts come later.
        # b_qkv loads contiguously as [18,128] (18 descriptors) and is PE-
        # transposed to [128,18] - the direct gather would be 2304 4-byte
        # descriptors hogging the DMA queues.
        bqk_raw = const_pool.tile([18, 128], f32, name="bqk_raw")
        nc.scalar.dma_start(bqk_raw[:], bqkv_d[:].rearrange("(m p) -> m p", m=18))
        bqk = const_pool.tile([128, 18], f32, name="bqk")
        bv_bc = const_pool.tile([128, C], f32, name="bv_bc")
        nc.scalar.dma_start(bv_bc[0:1, :], bqkv_d[2 * C : 3 * C][None, :])
        bo_bc = const_pool.tile([128, C], f32, name="bo_bc")
        nc.scalar.dma_start(bo_bc[0:1, :], bout_d[:][None, :])

        # x on the sync queue (first - the transposes gate everything), wv on
        # the scalar queue in parallel; transposes consume x as f32 directly.
        x_tiles = []
        for tb in range(TB):
            x_in = xin_pool.tile([128, C], f32, name="x_in", tag="x_in", bufs=TB)
            nc.sync.dma_start(x_in[:], x_d[tb * 128 : (tb + 1) * 128, :])
            x_tiles.append(x_in)
        wv_dma(0)
        wv_cast(0)
        nc.gpsimd.partition_broadcast(bv_bc[:], bv_bc[0:1, :])
        wv_dma(1)
        wv_cast(1)

        def issue_wqk(j):
            wqk = wqk_pool.tile([128, CB, 2, 128], bf16, name="wqk", tag="wqk")
            for qk in range(2):
                nc.gpsimd.dma_start(
                    wqk[:, :, qk, :],
                    wqkv_d[:, qk * C + j * 128 : qk * C + (j + 1) * 128].rearrange(
                        "(cb p) f -> p cb f", p=128
                    ),
                )
            return wqk

        wqk0 = issue_wqk(0)

        ones96 = const_pool.tile([128, TB * H], f32, name="ones96")
        nc.gpsimd.memset(ones96[:], 1.0)
        nc.vector.tensor_copy(
            V[:, :, :, D], ones96[:].rearrange("p (t h) -> p t h", t=TB)
        )
        nc.gpsimd.partition_broadcast(bo_bc[:], bo_bc[0:1, :])

        # wout DMA is deferred into the attention phase (the preamble is
        # HBM-bandwidth-bound; wout is not needed until the tail).
        wout = wres.tile([128, CB, C], bf16, name="wout")

        def issue_wout():
            nc.gpsimd.dma_start(
                wout[:, :, :], wout_d[:, :].rearrange("(cb p) f -> p cb f", p=128)
            )

        def proj_group_emitters(j, wqk, qkt):
            # one group per (qk, t-half): 6-mm f32r chain -> bias-add evac
            # casting to bf16 qkt. tch=0 groups only need xT t<512 (tb 0-3).
            ems = []
            for tch in range(2):
                for qk in range(2):
                    def g(qk=qk, tch=tch):
                        ps = st_psum.tile([128, 512], f32, name="ps_qk", tag="st")
                        for cb in range(CB):
                            nc.tensor.matmul(
                                ps[:],
                                wqk[:, cb, qk, :],
                                xT[:, cb, tch * 512 : (tch + 1) * 512],
                                start=(cb == 0),
                                stop=(cb == CB - 1),
                            )
                        m = qk * 6 + j
                        nc.vector.tensor_scalar_add(
                            qkt[:, qk, tch * 512 : (tch + 1) * 512],
                            ps[:],
                            bqk[:, m : m + 1],
                        )
                    ems.append(g)
            return ems

        # ---------- preamble: transpose x, project V, project Q/K pair 0 ----
        qkt = qkt_pool.tile([128, 2, T], bf16, name="qkt", tag="qkt")
        pre_projs = proj_group_emitters(0, wqk0, qkt)

        def v_chain(tb, ch):
            ps = o_psum.tile([128, 512], f32, name="ps_v", tag="ot")
            for cb in range(CB):
                nc.tensor.matmul(
                    ps[:, 0:384],
                    xT[:, cb, tb * 128 : (tb + 1) * 128],
                    wv[:, cb, ch * 384 : (ch + 1) * 384],
                    start=(cb == 0),
                    stop=(cb == CB - 1),
                )
            nc.vector.tensor_add(
                V[:, tb, ch * 6 : (ch + 1) * 6, 0:D],
                ps[:, 0:384].rearrange("p (h d) -> p h d", h=6),
                bv_bc[:, ch * 384 : (ch + 1) * 384].rearrange("p (h d) -> p h d", h=6),
            )

        for tb in range(TB):
            tpa = st_psum.tile([128, 512], f32, name="tpa", tag="st")
            for c in range(4):
                nc.tensor.transpose(
                    tpa[:, c * 128 : (c + 1) * 128],
                    x_tiles[tb][:, c * 128 : (c + 1) * 128],
                    ident[:],
                )
            tpb = st_psum.tile([128, 512], f32, name="tpb", tag="st")
            for c in range(2):
                nc.tensor.transpose(
                    tpb[:, c * 128 : (c + 1) * 128],
                    x_tiles[tb][:, (4 + c) * 128 : (5 + c) * 128],
                    ident[:],
                )
            nc.scalar.copy(
                xT[:, 0:4, tb * 128 : (tb + 1) * 128],
                tpa[:].rearrange("p (c t) -> p c t", c=4),
            )
            nc.vector.tensor_copy(
                xT[:, 4:6, tb * 128 : (tb + 1) * 128],
                tpb[:, 0:256].rearrange("p (c t) -> p c t", c=2),
            )
            v_chain(tb, 0)
            v_chain(tb, 1)
            if tb == 0:
                bps = st_psum.tile([128, 512], f32, name="bps", tag="st")
                nc.tensor.transpose(bps[:, 0:18], bqk_raw[:], ident[0:18, 0:18])
                nc.vector.tensor_copy(bqk[:], bps[:, 0:18])
            if tb == 3:
                pre_projs[0]()  # (t-half 0, q)
                pre_projs[1]()  # (t-half 0, k)
            if tb == 7:
                pre_projs[2]()
                pre_projs[3]()

        # ---------- attention ----------
        def norm(j, i, ot_qc, qc, variant):
            # OT[j][head-half, qc-chunk] = O'(0:D) * (1 / O'(D)) broadcast
            if variant == "act":
                lns = small_pool.tile([1, 512], f32, name="lns", tag="lns")
                nc.scalar.activation(lns[:], ot_qc[D : D + 1, :], Ln)
                recip = small_pool.tile([1, 512], f32, name="recip", tag="recip")
                nc.scalar.activation(recip[:], lns[:], Exp, scale=-1.0)
            else:
                dn = small_pool.tile([1, 512], f32, name="dn", tag="dn")
                nc.vector.tensor_copy(dn[:], ot_qc[D : D + 1, :])
                recip = small_pool.tile([1, 512], f32, name="recipd", tag="recipd")
                nc.vector.reciprocal_approx_fast(recip[:], dn[:])
            rbc = small_pool.tile([64, 512], f32, name="rbc", tag="rbc")
            nc.gpsimd.partition_broadcast(rbc[:], recip[:])
            nc.vector.tensor_mul(
                OT[j][i * 64 : (i + 1) * 64, qc * 512 : (qc + 1) * 512],
                ot_qc[0:D, :],
                rbc[:],
            )

        deferred = [None]
        for j in range(J):
            if j < J - 1:
                wqk_next = issue_wqk(j + 1)
                qkt_next = qkt_pool.tile([128, 2, T], bf16, name="qkt", tag="qkt")
                pending = proj_group_emitters(j + 1, wqk_next, qkt_next)
            else:
                pending = []
            if j == 2:
                issue_wout()

            # ---- pair-interleaved packed-S attention ----
            # Both heads of the pair advance in lockstep per 512-wide S chunk:
            # S_A and S_B are emitted adjacently so their K=64 matmuls run
            # CONCURRENTLY in separate PE row-groups (lhsT base partitions 0 /
            # 64 -> tile_position rows 0-63 / 64-127).  exp lags the S stream
            # by one chunk; PV lags by two full k-blocks; the last two PVs +
            # qc1 normalizes are deferred into the next pair.
            hA, hB = 2 * j, 2 * j + 1
            sts = {}  # (i, kb, hc) -> (st, a, b)
            pts = {}  # (i, kb) -> pt
            ots = {}  # i -> [ot_qc0, ot_qc1]

            def s_chunk(i, kb, hc, sts=sts, qkt=qkt):
                v0 = kb * 128
                a = max(v0, hc * 512)
                b = (hc + 1) * 512
                st = st_psum.tile([128, 512], f32, name="st", tag="st")
                nc.tensor.matmul(
                    st[:, a - hc * 512 : 512],
                    qkt[i * 64 : (i + 1) * 64, 1, v0 : v0 + 128],
                    qkt[i * 64 : (i + 1) * 64, 0, a:b],
                    start=True,
                    stop=True,
                )
                sts[(i, kb, hc)] = (st, a, b)

            def exp_chunk(i, kb, hc, sts=sts, pts=pts):
                v0 = kb * 128
                st, a, b = sts[(i, kb, hc)]
                diag_hc = 0 if kb < 4 else 1
                if hc == diag_hc:
                    pt = pt_pool.tile([128, T], bf16, name="pt", tag="pt")
                    pts[(i, kb)] = pt
                else:
                    pt = pts[(i, kb)]
                nc.scalar.activation(
                    pt[:, a:b], st[:, a - hc * 512 : b - hc * 512], Exp, scale=0.125
                )
                if hc == diag_hc:
                    nc.gpsimd.affine_select(
                        out=pt[:, v0 : v0 + 128],
                        in_=pt[:, v0 : v0 + 128],
                        compare_op=isge,
                        fill=0.0,
                        base=0,
                        channel_multiplier=-1,
                        pattern=[[1, 128]],
                    )

            def pv(i, pkb, last, j=j, ots=ots, pts=pts):
                pv0 = pkb * 128
                h = 2 * j + i
                ot = ots[i]
                for qc in range(pkb // 4, 2):
                    sq = max(pv0, qc * 512)
                    nc.tensor.matmul(
                        ot[qc][:, sq - qc * 512 : 512],
                        V[:, pkb, h, :],
                        pts[(i, pkb)][:, sq : (qc + 1) * 512],
                        start=(pkb == 0),
                        stop=(pkb == 3 + 4 * qc),
                    )
                if pkb == 3:
                    norm(j, i, ot[0], 0, "dve")
                if last:
                    norm(j, i, ot[1], 1, "dve")

            # chunk stream: kb<4 contributes (kb,0),(kb,1); kb>=4 only (kb,1)
            stream = []
            for kb in range(TB):
                if kb < 4:
                    stream.append((kb, 0))
                stream.append((kb, 1))

            for n, (kb, hc) in enumerate(stream):
                s_chunk(0, kb, hc)
                s_chunk(1, kb, hc)
                if n == 2 and deferred[0] is not None:
                    deferred[0]()
                    deferred[0] = None
                if n >= 1:
                    pkb, phc = stream[n - 1]
                    exp_chunk(0, pkb, phc)
                    exp_chunk(1, pkb, phc)
                first_of_kb = (hc == 0) if kb < 4 else True
                if first_of_kb and kb >= 2:
                    if kb == 2:
                        ots[0] = [
                            o_psum.tile([D + 1, 512], f32, name="otA", tag="ot")
                            for _ in range(2)
                        ]
                        ots[1] = [
                            o_psum.tile([D + 1, 512], f32, name="otB", tag="ot")
                            for _ in range(2)
                        ]
                    pv(0, kb - 2, last=False)
                    pv(1, kb - 2, last=False)
                if n in (2, 5, 8, 10) and pending:
                    pending.pop(0)()

            # pair tail: exp the final chunk now; PV(6), PV(7) + qc1
            # normalizes deferred into the next pair's pipeline.
            exp_chunk(0, 7, 1)
            exp_chunk(1, 7, 1)

            def make_deferred(pv=pv):
                def d():
                    pv(0, 6, last=False)
                    pv(1, 6, last=False)
                    pv(0, 7, last=True)
                    pv(1, 7, last=True)
                return d

            deferred[0] = make_deferred()

            for g in pending:
                g()
            if j < J - 1:
                qkt = qkt_next

        if deferred[0] is not None:
            deferred[0]()
            deferred[0] = None

        # ---------- output projection ----------
        for tb in range(TB):
            yt = y_pool.tile([128, C], f32, name="yt", tag="yt")
            for ch in range(2):
                ps = o_psum.tile([128, 512], f32, name="ps_y", tag="ot")
                for cb in range(CB):
                    nc.tensor.matmul(
                        ps[:, 0:384],
                        OT[cb][:, tb * 128 : (tb + 1) * 128],
                        wout[:, cb, ch * 384 : (ch + 1) * 384],
                        start=(cb == 0),
                        stop=(cb == CB - 1),
                    )
                nc.vector.tensor_add(
                    yt[:, ch * 384 : (ch + 1) * 384],
                    ps[:, 0:384],
                    bo_bc[:, ch * 384 : (ch + 1) * 384],
                )
            nc.sync.dma_start(y_d[tb * 128 : (tb + 1) * 128, :], yt[:])


def build():
    if "nc" in _CACHE:
        return _CACHE["nc"]
    _ensure_path()
    import concourse.bacc as bacc
    import concourse.mybir as mybir
    import concourse.tile as tile
    from concourse.masks import make_identity

    nc = bacc.Bacc(
        "TRN2",
        target_bir_lowering=False,
        debug=False,
        enable_asserts=False,
        num_devices=NCORES,
    )
    with tile.TileContext(nc) as tc:
        _emit(nc, tc, tile, mybir, make_identity)

    # Both Exp and Ln live in the 'natural_log_exp_and_others' ACT table set,
    # but the table-load pass maps Exp to the first set containing it
    # ('exp_and_others'), so Exp/Ln ping-pong table loads every head
    # (~1.3us each).  Restrict Exp membership to the natural_log set for the
    # duration of compile; dict order (= act_func_set_id) is preserved.
    orig_tables = bacc.get_activation_tables

    def _pinned_tables(arch):
        tables = orig_tables(arch)
        exp_t = mybir.ActivationFunctionType.Exp
        if any(exp_t in fns for name, fns in tables.items() if "natural_log" in name):
            for name, fns in tables.items():
                if "natural_log" not in name:
                    fns.discard(exp_t)
        return tables

    bacc.get_activation_tables = _pinned_tables
    try:
        nc.compile()
    finally:
        bacc.get_activation_tables = orig_tables
    _CACHE["nc"] = nc
    return nc


def _in_maps(x, W_qkv, b_qkv, W_out, b_out):
    x = np.ascontiguousarray(np.asarray(x, dtype=np.float32))
    W_qkv = np.ascontiguousarray(np.asarray(W_qkv, dtype=np.float32))
    b_qkv = np.ascontiguousarray(np.asarray(b_qkv, dtype=np.float32))
    W_out = np.ascontiguousarray(np.asarray(W_out, dtype=np.float32))
    b_out = np.ascontiguousarray(np.asarray(b_out, dtype=np.float32))
    return [
        {
            "x": x[b],
            "W_qkv": W_qkv,
            "b_qkv": b_qkv,
            "W_out": W_out,
            "b_out": b_out,
        }
        for b in range(B)
    ]


def _install_ntff_hook():
    """The image's antenv package lacks axon_hooks; synthesize it so
    run_bass_kernel_spmd(trace=True) can NTFF-profile via libaxon_pjrt.so."""
    import sys
    import types

    if "antenv.axon_hooks" in sys.modules:
        return
    mod = types.ModuleType("antenv.axon_hooks")
    state = {"hook": None}
    mod.set_axon_ntff_profile_hook = lambda h: state.__setitem__("hook", h)
    mod.get_axon_ntff_profile_hook = lambda: state["hook"]
    sys.modules["antenv.axon_hooks"] = mod
    import antenv

    antenv.axon_hooks = mod
    try:
        if "/root/.axon_site" not in sys.path:
            sys.path.append("/root/.axon_site")
        from trn_agent_boot.trn_boot import _ntff_profile_via_ctypes

        mod.set_axon_ntff_profile_hook(
            _ntff_profile_via_ctypes("/opt/axon/libaxon_pjrt.so")
        )
    except Exception as exc:  # degrade to no tracing
        print(f"ntff hook unavailable: {exc}", file=sys.stderr)


def run(x, W_qkv, b_qkv, W_out, b_out, trace=False):
    _ensure_path()
    if trace:
        _install_ntff_hook()
    from concourse.bass_utils import run_bass_kernel_spmd

    nc = build()
    res = run_bass_kernel_spmd(
        nc,
        _in_maps(x, W_qkv, b_qkv, W_out, b_out),
        core_ids=list(range(NCORES)),
        trace=trace,
    )
    y = np.stack([res.results[b]["y_out"] for b in range(B)], axis=0)
    return y.astype(np.float32, copy=False), res


def kernel(x, W_qkv, b_qkv, W_out, b_out):
    y, _ = run(x, W_qkv, b_qkv, W_out, b_out, trace=False)
    return y


# revision 18
# speedup vs baseline: 1.0588x; 1.0588x over previous
"""Causal self-attention (B=8, T=1024, C=768, H=12, Dh=64) on 8 TRN2 NeuronCores.

Sharding: batch data-parallel. Core b computes the full attention block for
batch element b (weights replicated). No collectives.

Per-core dataflow (all matmuls bf16; fp32 weights cast on idle engines so
every stationary operand gets fast bf16 LDWEIGHTS/FWL):
  1. x [T,C] -> bf16 (DVE cast) -> xT [C,T] via PE transposes, ACT evac.
  2. Q^T,K^T [C,T] = W^T @ xT (evac casts to bf16 qkt); V [t, c] = x @ W_v
     (bias-add evac to bf16), stored per head with an all-ones column
     (V_aug [k, 65]) so the P@V matmul also accumulates softmax denominators.
     W_out DMA+cast deferred to mid-attention (preamble is HBM-bound).
  3. Per head h, software-pipelined over k-blocks: S^T(kb) [k=128, q] emitted
     ahead; P^T = exp(S^T/8) (ACT, bf16 out, exact causal spans) one block
     behind; sub-diagonal zeroing via in-place gpsimd affine_select; PV
     (O'^T [65, q] += V_aug^T @ P^T) lags TWO blocks so its mask is always
     ready when the PE reaches it. Row 64 of O' = softmax denominator.
  4. Normalize per 512-wide q-chunk: qc0 on ACT (1/s = exp(-ln s), same pinned
     table), qc1 on DVE (reciprocal_approx_fast); gpsimd partition_broadcast;
     DVE multiply -> OT (bf16). The last two PVs + qc1 normalize of each head
     are deferred into the next head's pipeline.
  5. y [T,C] = OT-as-lhsT @ W_out + b_out, DMA to DRAM.
"""

import numpy as np

B, T, C = 8, 1024, 768
H, D = 12, 64
TB = T // 128  # 8 t/k blocks
CB = C // 128  # 6 channel blocks
J = H // 2  # 6 head pairs
NCORES = 8

_CACHE = {}


def _ensure_path():
    import sys

    for p in ("/opt/trn_rl_repo",):
        if p not in sys.path:
            sys.path.insert(0, p)


def _emit(nc, tc, tile, mybir, make_identity):
    f32 = mybir.dt.float32
    f32r = mybir.dt.float32r
    bf16 = mybir.dt.bfloat16
    Exp = mybir.ActivationFunctionType.Exp
    Ln = mybir.ActivationFunctionType.Ln
    isge = mybir.AluOpType.is_ge

    x_d = nc.dram_tensor("x", [T, C], f32, kind="ExternalInput")
    wqkv_d = nc.dram_tensor("W_qkv", [C, 3 * C], f32r, kind="ExternalInput")
    bqkv_d = nc.dram_tensor("b_qkv", [3 * C], f32, kind="ExternalInput")
    wout_d = nc.dram_tensor("W_out", [C, C], f32r, kind="ExternalInput")
    bout_d = nc.dram_tensor("b_out", [C], f32, kind="ExternalInput")
    y_d = nc.dram_tensor("y_out", [T, C], f32, kind="ExternalOutput")

    with (
        tc.tile_pool(name="const", bufs=1) as const_pool,
        tc.tile_pool(name="wres", bufs=1) as wres,
        tc.tile_pool(name="wqkp", bufs=2) as wqk_pool,
        tc.tile_pool(name="xin", bufs=4) as xin_pool,
        tc.tile_pool(name="big", bufs=1) as big,
        tc.tile_pool(name="qktp", bufs=2) as qkt_pool,
        tc.tile_pool(name="ptp", bufs=6) as pt_pool,
        tc.tile_pool(name="yp", bufs=2) as y_pool,
        tc.tile_pool(name="smallp", bufs=2) as small_pool,
        # PSUM: st 4x[128,512]f32 = 4 banks (S chunks + proj chains +
        # transposes share the rotation); ot 4x[*,512]f32 = 4 banks.
        tc.tile_pool(name="stp", bufs=4, space="PSUM") as st_psum,
        tc.tile_pool(name="op", bufs=4, space="PSUM") as o_psum,
    ):
        ident = const_pool.tile([128, 128], f32, name="ident")
        make_identity(nc, ident[:])

        # x and W_v load at full HWDGE rate as f32 and are cast to bf16 on
        # the (preamble-idle) DVE / ACT engines; wqk and wout use gpsimd
        # SWDGE casting DMAs (lower DMA rate, but off the critical path).
        wv_raw = wres.tile([128, CB, C], f32r, name="wv_raw")
        wv = wres.tile([128, CB, C], bf16, name="wv")

        def wv_dma(half):
            for cb in range(CB):
                nc.sync.dma_start(
                    wv_raw[:, cb, half * 384 : (half + 1) * 384],
                    wqkv_d[
                        cb * 128 : (cb + 1) * 128,
                        2 * C + half * 384 : 2 * C + (half + 1) * 384,
                    ],
                )

        def wv_cast(half):
            for cb in range(CB):
                nc.scalar.copy(
                    wv[:, cb, half * 384 : (half + 1) * 384],
                    wv_raw[:, cb, half * 384 : (half + 1) * 384].bitcast(f32),
                )

        xT = big.tile([128, CB, T], bf16, name="xT")
        V = big.tile([128, TB, H, D + 1], bf16, name="V")
        OT = [big.tile([128, T], bf16, name=f"OT{j}", tag=f"OT{j}") for j in range(J)]

        # bias DMAs early (HWDGE, cheap); their gpsimd broadcasts come later.
        # b_qkv loads contiguously as [18,128] (18 descriptors) and is PE-
        # transposed to [128,18] - the direct gather would be 2304 4-byte
        # descriptors hogging the DMA queues.
        bqk_raw = const_pool.tile([18, 128], f32, name="bqk_raw")
        nc.scalar.dma_start(bqk_raw[:], bqkv_d[:].rearrange("(m p) -> m p", m=18))
        bqk = const_pool.tile([128, 18], f32, name="bqk")
        bv_bc = const_pool.tile([128, C], f32, name="bv_bc")
        nc.scalar.dma_start(bv_bc[0:1, :], bqkv_d[2 * C : 3 * C][None, :])
        bo_bc = const_pool.tile([128, C], f32, name="bo_bc")
        nc.scalar.dma_start(bo_bc[0:1, :], bout_d[:][None, :])

        # x on the sync queue (first - the transposes gate everything), wv on
        # the scalar queue in parallel; transposes consume x as f32 directly.
        x_tiles = []
        for tb in range(TB):
            x_in = xin_pool.tile([128, C], f32, name="x_in", tag="x_in", bufs=TB)
            nc.sync.dma_start(x_in[:], x_d[tb * 128 : (tb + 1) * 128, :])
            x_tiles.append(x_in)
        wv_dma(0)
        wv_cast(0)
        nc.gpsimd.partition_broadcast(bv_bc[:], bv_bc[0:1, :])
        wv_dma(1)
        wv_cast(1)

        def issue_wqk(j):
            wqk = wqk_pool.tile([128, CB, 2, 128], bf16, name="wqk", tag="wqk")
            for qk in range(2):
                nc.gpsimd.dma_start(
                    wqk[:, :, qk, :],
                    wqkv_d[:, qk * C + j * 128 : qk * C + (j + 1) * 128].rearrange(
                        "(cb p) f -> p cb f", p=128
                    ),
                )
            return wqk

        wqk0 = issue_wqk(0)

        ones96 = const_pool.tile([128, TB * H], f32, name="ones96")
        nc.gpsimd.memset(ones96[:], 1.0)
        nc.vector.tensor_copy(
            V[:, :, :, D], ones96[:].rearrange("p (t h) -> p t h", t=TB)
        )
        nc.gpsimd.partition_broadcast(bo_bc[:], bo_bc[0:1, :])

        # wout DMA is deferred into the attention phase (the preamble is
        # HBM-bandwidth-bound; wout is not needed until the tail).
        wout = wres.tile([128, CB, C], bf16, name="wout")

        def issue_wout():
            nc.gpsimd.dma_start(
                wout[:, :, :], wout_d[:, :].rearrange("(cb p) f -> p cb f", p=128)
            )

        def proj_group_emitters(j, wqk, qkt):
            # one group per (qk, t-half): 6-mm f32r chain -> bias-add evac
            # casting to bf16 qkt. tch=0 groups only need xT t<512 (tb 0-3).
            ems = []
            for tch in range(2):
                for qk in range(2):
                    def g(qk=qk, tch=tch):
                        ps = st_psum.tile([128, 512], f32, name="ps_qk", tag="st")
                        for cb in range(CB):
                            nc.tensor.matmul(
                                ps[:],
                                wqk[:, cb, qk, :],
                                xT[:, cb, tch * 512 : (tch + 1) * 512],
                                start=(cb == 0),
                                stop=(cb == CB - 1),
                            )
                        m = qk * 6 + j
                        nc.vector.tensor_scalar_add(
                            qkt[:, qk, tch * 512 : (tch + 1) * 512],
                            ps[:],
                            bqk[:, m : m + 1],
                        )
                    ems.append(g)
            return ems

        # ---------- preamble: transpose x, project V, project Q/K pair 0 ----
        qkt = qkt_pool.tile([128, 2, T], bf16, name="qkt", tag="qkt")
        pre_projs = proj_group_emitters(0, wqk0, qkt)

        def v_chain(tb, ch):
            ps = o_psum.tile([128, 512], f32, name="ps_v", tag="ot")
            for cb in range(CB):
                nc.tensor.matmul(
                    ps[:, 0:384],
                    xT[:, cb, tb * 128 : (tb + 1) * 128],
                    wv[:, cb, ch * 384 : (ch + 1) * 384],
                    start=(cb == 0),
                    stop=(cb == CB - 1),
                )
            nc.vector.tensor_add(
                V[:, tb, ch * 6 : (ch + 1) * 6, 0:D],
                ps[:, 0:384].rearrange("p (h d) -> p h d", h=6),
                bv_bc[:, ch * 384 : (ch + 1) * 384].rearrange("p (h d) -> p h d", h=6),
            )

        for tb in range(TB):
            tpa = st_psum.tile([128, 512], f32, name="tpa", tag="st")
            for c in range(4):
                nc.tensor.transpose(
                    tpa[:, c * 128 : (c + 1) * 128],
                    x_tiles[tb][:, c * 128 : (c + 1) * 128],
                    ident[:],
                )
            tpb = st_psum.tile([128, 512], f32, name="tpb", tag="st")
            for c in range(2):
                nc.tensor.transpose(
                    tpb[:, c * 128 : (c + 1) * 128],
                    x_tiles[tb][:, (4 + c) * 128 : (5 + c) * 128],
                    ident[:],
                )
            nc.scalar.copy(
                xT[:, 0:4, tb * 128 : (tb + 1) * 128],
                tpa[:].rearrange("p (c t) -> p c t", c=4),
            )
            nc.vector.tensor_copy(
                xT[:, 4:6, tb * 128 : (tb + 1) * 128],
                tpb[:, 0:256].rearrange("p (c t) -> p c t", c=2),
            )
            v_chain(tb, 0)
            v_chain(tb, 1)
            if tb == 0:
                bps = st_psum.tile([128, 512], f32, name="bps", tag="st")
                nc.tensor.transpose(bps[:, 0:18], bqk_raw[:], ident[0:18, 0:18])
                nc.vector.tensor_copy(bqk[:], bps[:, 0:18])
            if tb == 3:
                pre_projs[0]()  # (t-half 0, q)
                pre_projs[1]()  # (t-half 0, k)
            if tb == 7:
                pre_projs[2]()
                pre_projs[3]()

        # ---------- attention ----------
        def norm(j, i, ot_qc, qc, variant):
            # OT[j][head-half, qc-chunk] = O'(0:D) * (1 / O'(D)) broadcast
            if variant == "act":
                lns = small_pool.tile([1, 512], f32, name="lns", tag="lns")
                nc.scalar.activation(lns[:], ot_qc[D : D + 1, :], Ln)
                recip = small_pool.tile([1, 512], f32, name="recip", tag="recip")
                nc.scalar.activation(recip[:], lns[:], Exp, scale=-1.0)
            else:
                dn = small_pool.tile([1, 512], f32, name="dn", tag="dn")
                nc.vector.tensor_copy(dn[:], ot_qc[D : D + 1, :])
                recip = small_pool.tile([1, 512], f32, name="recipd", tag="recipd")
                nc.vector.reciprocal_approx_fast(recip[:], dn[:])
            rbc = small_pool.tile([64, 512], f32, name="rbc", tag="rbc")
            nc.gpsimd.partition_broadcast(rbc[:], recip[:])
            nc.vector.tensor_mul(
                OT[j][i * 64 : (i + 1) * 64, qc * 512 : (qc + 1) * 512],
                ot_qc[0:D, :],
                rbc[:],
            )

        deferred = [None]
        for j in range(J):
            if j < J - 1:
                wqk_next = issue_wqk(j + 1)
                qkt_next = qkt_pool.tile([128, 2, T], bf16, name="qkt", tag="qkt")
                pending = proj_group_emitters(j + 1, wqk_next, qkt_next)
            else:
                pending = []
            if j == 2:
                issue_wout()

            # ---- pair-interleaved packed-S attention ----
            # Both heads of the pair advance in lockstep per 512-wide S chunk:
            # S_A and S_B are emitted adjacently so their K=64 matmuls run
            # CONCURRENTLY in separate PE row-groups (lhsT base partitions 0 /
            # 64 -> tile_position rows 0-63 / 64-127).  exp lags the S stream
            # by one chunk; PV lags by two full k-blocks; the last two PVs +
            # qc1 normalizes are deferred into the next pair.
            hA, hB = 2 * j, 2 * j + 1
            sts = {}  # (i, kb, hc) -> (st, a, b)
            pts = {}  # (i, kb) -> pt
            ots = {}  # i -> [ot_qc0, ot_qc1]

            def s_chunk(i, kb, hc, sts=sts, qkt=qkt):
                v0 = kb * 128
                a = max(v0, hc * 512)
                b = (hc + 1) * 512
                st = st_psum.tile([128, 512], f32, name="st", tag="st")
                nc.tensor.matmul(
                    st[:, a - hc * 512 : 512],
                    qkt[i * 64 : (i + 1) * 64, 1, v0 : v0 + 128],
                    qkt[i * 64 : (i + 1) * 64, 0, a:b],
                    start=True,
                    stop=True,
                )
                sts[(i, kb, hc)] = (st, a, b)

            def exp_chunk(i, kb, hc, sts=sts, pts=pts):
                v0 = kb * 128
                st, a, b = sts[(i, kb, hc)]
                diag_hc = 0 if kb < 4 else 1
                if hc == diag_hc:
                    pt = pt_pool.tile([128, T], bf16, name="pt", tag="pt")
                    pts[(i, kb)] = pt
                else:
                    pt = pts[(i, kb)]
                nc.scalar.activation(
                    pt[:, a:b], st[:, a - hc * 512 : b - hc * 512], Exp, scale=0.125
                )
                if hc == diag_hc:
                    nc.gpsimd.affine_select(
                        out=pt[:, v0 : v0 + 128],
                        in_=pt[:, v0 : v0 + 128],
                        compare_op=isge,
                        fill=0.0,
                        base=0,
                        channel_multiplier=-1,
                        pattern=[[1, 128]],
                    )

            def pv(i, pkb, last, j=j, ots=ots, pts=pts):
                pv0 = pkb * 128
                h = 2 * j + i
                ot = ots[i]
                for qc in range(pkb // 4, 2):
                    sq = max(pv0, qc * 512)
                    nc.tensor.matmul(
                        ot[qc][:, sq - qc * 512 : 512],
                        V[:, pkb, h, :],
                        pts[(i, pkb)][:, sq : (qc + 1) * 512],
                        start=(pkb == 0),
                        stop=(pkb == 3 + 4 * qc),
                    )
                if pkb == 3:
                    norm(j, i, ot[0], 0, "dve")
                if last:
                    norm(j, i, ot[1], 1, "dve")

            # chunk stream: kb<4 contributes (kb,0),(kb,1); kb>=4 only (kb,1)
            stream = []
            for kb in range(TB):
                if kb < 4:
                    stream.append((kb, 0))
                stream.append((kb, 1))

            for n, (kb, hc) in enumerate(stream):
                s_chunk(0, kb, hc)
                s_chunk(1, kb, hc)
                if n == 2 and deferred[0] is not None:
                    deferred[0]()
                    deferred[0] = None
                if n >= 1:
                    pkb, phc = stream[n - 1]
                    exp_chunk(0, pkb, phc)
                    exp_chunk(1, pkb, phc)
                first_of_kb = (hc == 0) if kb < 4 else True
                if first_of_kb and kb >= 2:
                    if kb == 2:
                        ots[0] = [
                            o_psum.tile([D + 1, 512], f32, name="otA", tag="ot")
                            for _ in range(2)
                        ]
                        ots[1] = [
                            o_psum.tile([D + 1, 512], f32, name="otB", tag="ot")
                            for _ in range(2)
                        ]
                    pv(0, kb - 2, last=False)
                    pv(1, kb - 2, last=False)
                if n in (2, 5, 8, 10) and pending:
                    pending.pop(0)()

            # pair tail: exp the final chunk now; PV(6), PV(7) + qc1
            # normalizes deferred into the next pair's pipeline.
            exp_chunk(0, 7, 1)
            exp_chunk(1, 7, 1)

            def make_deferred(pv=pv):
                def d():
                    pv(0, 6, last=False)
                    pv(1, 6, last=False)
                    pv(0, 7, last=True)
                    pv(1, 7, last=True)
                return d

            deferred[0] = make_deferred()

            for g in pending:
                g()
            if j < J - 1:
                qkt = qkt_next

        if deferred[0] is not None:
            deferred[0]()
            deferred[0] = None

        # ---------- output projection ----------
        for tb in range(TB):
            yt = y_pool.tile([128, C], f32, name="yt", tag="yt")
            for ch in range(2):
                ps = o_psum.tile([128, 512], f32, name="ps_y", tag="ot")
                for cb in range(CB):
                    nc.tensor.matmul(
                        ps[:, 0:384],
                        OT[cb][:, tb * 128 : (tb + 1) * 128],
                        wout[:, cb, ch * 384 : (ch + 1) * 384],
                        start=(cb == 0),
                        stop=(cb == CB - 1),
                    )
                nc.vector.tensor_add(
                    yt[:, ch * 384 : (ch + 1) * 384],
                    ps[:, 0:384],
                    bo_bc[:, ch * 384 : (ch + 1) * 384],
                )
            nc.sync.dma_start(y_d[tb * 128 : (tb + 1) * 128, :], yt[:])


def build():
    if "nc" in _CACHE:
        return _CACHE["nc"]
    _ensure_path()
    import concourse.bacc as bacc
    import concourse.mybir as mybir
    import concourse.tile as tile
    from concourse.masks import make_identity

    nc = bacc.Bacc(
        "TRN2",
        target_bir_lowering=False,
        debug=False,
        enable_asserts=False,
        num_devices=NCORES,
    )
    with tile.TileContext(nc) as tc:
        _emit(nc, tc, tile, mybir, make_identity)

    # Both Exp and Ln live in the 'natural_log_exp_and_others' ACT table set,
    # but the table-load pass maps Exp to the first set containing it
    # ('exp_and_others'), so Exp/Ln ping-pong table loads every head
    # (~1.3us each).  Restrict Exp membership to the natural_log set for the
    # duration of compile; dict order (= act_func_set_id) is preserved.
    orig_tables = bacc.get_activation_tables

    def _pinned_tables(arch):
        tables = orig_tables(arch)
        exp_t = mybir.ActivationFunctionType.Exp
        if any(exp_t in fns for name, fns in tables.items() if "natural_log" in name):
            for name, fns in tables.items():
                if "natural_log" not in name:
                    fns.discard(exp_t)
        return tables

    bacc.get_activation_tables = _pinned_tables
    try:
        nc.compile()
    finally:
        bacc.get_activation_tables = orig_tables
    _CACHE["nc"] = nc
    return nc


def _in_maps(x, W_qkv, b_qkv, W_out, b_out):
    x = np.ascontiguousarray(np.asarray(x, dtype=np.float32))
    W_qkv = np.ascontiguousarray(np.asarray(W_qkv, dtype=np.float32))
    b_qkv = np.ascontiguousarray(np.asarray(b_qkv, dtype=np.float32))
    W_out = np.ascontiguousarray(np.asarray(W_out, dtype=np.float32))
    b_out = np.ascontiguousarray(np.asarray(b_out, dtype=np.float32))
    return [
        {
            "x": x[b],
            "W_qkv": W_qkv,
            "b_qkv": b_qkv,
            "W_out": W_out,
            "b_out": b_out,
        }
        for b in range(B)
    ]


def _install_ntff_hook():
    """The image's antenv package lacks axon_hooks; synthesize it so
    run_bass_kernel_spmd(trace=True) can NTFF-profile via libaxon_pjrt.so."""
    import sys
    import types

    if "antenv.axon_hooks" in sys.modules:
        return
    mod = types.ModuleType("antenv.axon_hooks")
    state = {"hook": None}
    mod.set_axon_ntff_profile_hook = lambda h: state.__setitem__("hook", h)
    mod.get_axon_ntff_profile_hook = lambda: state["hook"]
    sys.modules["antenv.axon_hooks"] = mod
    import antenv

    antenv.axon_hooks = mod
    try:
        if "/root/.axon_site" not in sys.path:
            sys.path.append("/root/.axon_site")
        from trn_agent_boot.trn_boot import _ntff_profile_via_ctypes

        mod.set_axon_ntff_profile_hook(
            _ntff_profile_via_ctypes("/opt/axon/libaxon_pjrt.so")
        )
    except Exception as exc:  # degrade to no tracing
        print(f"ntff hook unavailable: {exc}", file=sys.stderr)


def run(x, W_qkv, b_qkv, W_out, b_out, trace=False):
    _ensure_path()
    if trace:
        _install_ntff_hook()
    from concourse.bass_utils import run_bass_kernel_spmd

    nc = build()
    res = run_bass_kernel_spmd(
        nc,
        _in_maps(x, W_qkv, b_qkv, W_out, b_out),
        core_ids=list(range(NCORES)),
        trace=trace,
    )
    y = np.stack([res.results[b]["y_out"] for b in range(B)], axis=0)
    return y.astype(np.float32, copy=False), res


def kernel(x, W_qkv, b_qkv, W_out, b_out):
    y, _ = run(x, W_qkv, b_qkv, W_out, b_out, trace=False)
    return y
